# revision 5
# baseline (speedup 1.0000x reference)
"""Trainium2 Bass kernel for DepthwiseTensorProductModuleDict.

Computes, for each key k in {a, b}:
    w = MLP(edge_len_k)           # Linear(64->128) -> LayerNorm -> silu -> Linear(128->256)
    out_k = DTP(edge_fea_k, edge_vec_k, w)   # depthwise uvu tensor product

Sharding: edge dimension split across 8 NeuronCores (pure data parallel),
both dict keys processed by every core on its edge shard. Weights replicated.

Layout: edges packed 4 per partition -> macro tiles of 512 edges
[128 partitions, 4 slots, features]. Per-macro pipeline:
  PE: transpose len -> mm1 (fp32, N=129 with fused mean column) ->
      transpose a -> mm2 (float32r, N=384, host-packed [w1|w2|w3rep|w4])
  ACT: Square+accum (sum h^2), Silu(scale,bias) for layernorm+silu fusion,
       PSUM->SBUF copies (incl. float32r rounding for mm2 operands)
  DVE/GPSIMD: depthwise tensor product elementwise ops
"""
import os
import numpy as np

import concourse.bass as bass
import concourse.tile as tile
from concourse import bacc, mybir
from concourse.bass_utils import run_bass_kernel_spmd
from concourse.masks import make_identity

F32 = mybir.dt.float32
F32R = mybir.dt.float32r
I32 = mybir.dt.int32
P = 128          # partitions
J = 4            # edges per partition
MACRO = P * J    # 512 edges per macro tile
E = 131072       # total edges per key
NCORE = 8
ESH = E // NCORE          # 16384 edges per core per key
NM = ESH // MACRO         # 32 macros per key per core
MUL = 64
FEA = 256
RAD = 64
HID = 128
EPS = 1e-5

_mult = mybir.AluOpType.mult
_add = mybir.AluOpType.add
_sub = mybir.AluOpType.subtract

# cached compiled program (host-side) keyed by (b1_nz, gbe_nz) per key
_CACHE = {}

last_exec_time_ns = None
last_results = None


def _prep_weights(W1, b1, W2):
    """Host-side weight packing.

    Returns W1m [64 or 65, 129], W2big [128, 384], b1_nz flag.
    W1m = [W1; b1?] with extra column = rowwise mean weights (mu fused in mm1).
    W2big columns: [w1*s2 | w2*s2 | w3rep*s2 (each col x3 interleaved) | w4*s2*s3]
    where s2 = 1/sqrt(2), s3 = 1/sqrt(3).
    """
    inv_s2 = np.float32(1.0 / np.sqrt(np.float32(2.0)))
    inv_s3 = np.float32(1.0 / np.sqrt(np.float32(3.0)))
    b1_nz = bool(np.any(b1))
    Wstack = np.vstack([W1, b1[None, :]]) if b1_nz else W1   # [64(65), 128]
    mu_col = Wstack.mean(axis=1, keepdims=True)              # [*, 1]
    pad = np.zeros_like(mu_col)
    # [*, 130]: col 128 = mean weights, col 129 = zero pad (fp32r needs even N)
    W1m = np.hstack([Wstack, mu_col, pad]).astype(np.float32)

    w1 = W2[:, 0:64] * inv_s2
    w2 = W2[:, 64:128] * inv_s2
    w3 = W2[:, 128:192] * inv_s2
    w4 = W2[:, 192:256] * (inv_s2 * inv_s3)
    w3rep = np.repeat(w3, 3, axis=1)                         # [128, 192]
    W2big = np.concatenate([w1, w2, w3rep, w4], axis=1).astype(np.float32)
    return W1m, W2big, b1_nz


def _build_key(nc, tc, ctx, key, b1_nz, gbe_nz, ident, ident_r, magic4, pools):
    """Emit instructions for one dict key's full shard (NM macros)."""
    KROWS = 65 if b1_nz else 64

    fea = nc.dram_tensor(f"fea_{key}", [ESH, FEA], F32, kind="ExternalInput").ap()
    vec = nc.dram_tensor(f"vec_{key}", [ESH, 4], F32, kind="ExternalInput").ap()
    lng = nc.dram_tensor(f"len_{key}", [ESH, RAD], F32, kind="ExternalInput").ap()
    w1m_d = nc.dram_tensor(f"w1m_{key}", [KROWS, HID + 2], F32,
                           kind="ExternalInput").ap()
    w2b_d = nc.dram_tensor(f"w2b_{key}", [HID, 384], F32,
                           kind="ExternalInput").ap()
    out = nc.dram_tensor(f"out_{key}", [ESH, FEA], F32, kind="ExternalOutput").ap()
    g_d = be_d = None
    if gbe_nz:
        g_d = nc.dram_tensor(f"g_{key}", [HID], F32, kind="ExternalInput").ap()
        be_d = nc.dram_tensor(f"be_{key}", [HID], F32, kind="ExternalInput").ap()

    fea_v = fea.rearrange("(m p j) f -> m p j f", p=P, j=J)
    len_v = lng.rearrange("(m p j) f -> m p j f", p=P, j=J)
    out_v = out.rearrange("(m p j) f -> m p j f", p=P, j=J)
    vec_v = vec.rearrange("(m p j) f -> p m (j f)", p=P, j=J)   # [128, NM, 16]

    const = ctx.enter_context(tc.tile_pool(name=f"const_{key}", bufs=1))

    # --- weights ---
    w1m_stage = const.tile([KROWS, HID + 2], F32)
    nc.sync.dma_start(out=w1m_stage, in_=w1m_d)
    w1m_sb = const.tile([KROWS, HID + 2], F32R)
    nc.scalar.copy(w1m_sb, w1m_stage)
    w2stage = const.tile([HID, 384], F32)
    nc.sync.dma_start(out=w2stage, in_=w2b_d)
    w2r = const.tile([HID, 384], F32R)
    nc.scalar.copy(w2r, w2stage)

    grep_sb = berep_sb = None
    if gbe_nz:
        grep_sb = const.tile([P, HID], F32)
        berep_sb = const.tile([P, HID], F32)
        nc.sync.dma_start(out=grep_sb, in_=g_d.partition_broadcast(P))
        nc.sync.dma_start(out=berep_sb, in_=be_d.partition_broadcast(P))

    # --- whole-shard vec resident in SBUF ---
    vec_sb = const.tile([P, NM, J * 4], F32)
    nc.sync.dma_start(out=vec_sb, in_=vec_v)

    io, wk, st, ps_lt, ps_h, ps_at, ps_w = pools

    for m in range(NM):
        # ---------- loads ----------
        len_t = io.tile([P, J, RAD], F32, name="len_t")
        nc.sync.dma_start(out=len_t, in_=len_v[m])
        fea_t = io.tile([P, J, FEA], F32, name="fea_t")
        nc.sync.dma_start(out=fea_t, in_=fea_v[m])

        # ---------- PE front: transpose len, mm1 ----------
        lt_ps = ps_lt.tile([RAD, J * P], F32, name="lt_ps")
        for j in range(J):
            nc.tensor.transpose(lt_ps[:, j * P:(j + 1) * P], len_t[:, j, :], ident)
        lt_sb = wk.tile([KROWS, J * P], F32R, name="lt_sb")
        nc.scalar.copy(lt_sb[0:RAD, :], lt_ps)
        if b1_nz:
            nc.gpsimd.memset(lt_sb[RAD:KROWS, :], 1.0)

        h_ps = ps_h.tile([P, J, 256], F32, name="h_ps")
        for j in range(J):
            nc.tensor.matmul(h_ps[:, j, 0:HID + 2],
                             lt_sb[:, j * P:(j + 1) * P], w1m_sb,
                             start=True, stop=True)

        # ---------- layernorm stats ----------
        sq_d = wk.tile([P, J, HID], F32, name="sq_d")
        ssq = st.tile([P, J], F32, name="ssq")
        for j in range(J):
            nc.scalar.activation(sq_d[:, j], h_ps[:, j, 0:HID],
                                 mybir.ActivationFunctionType.Square,
                                 accum_out=ssq[:, j:j + 1])
        mus = st.tile([P, J], F32, name="mus")
        nc.vector.tensor_copy(mus, h_ps[:, :, HID:HID + 1].squeeze(2))

        # var = ssq/128 - mus^2 ; rstd = 1/sqrt(var+eps); nbias = -mus*rstd
        musq = st.tile([P, J], F32, name="musq")
        nc.gpsimd.tensor_tensor(out=musq, in0=mus, in1=mus, op=_mult)
        var = st.tile([P, J], F32, name="var")
        nc.vector.scalar_tensor_tensor(out=var, in0=ssq, scalar=1.0 / HID,
                                       in1=musq, op0=_mult, op1=_sub)
        vpe = st.tile([P, J], F32, name="vpe")
        nc.vector.tensor_scalar(out=vpe, in0=var, scalar1=EPS, scalar2=None,
                                op0=_add)
        nvpe = st.tile([P, J], F32, name="nvpe")
        nc.vector.tensor_scalar(out=nvpe, in0=vpe, scalar1=-0.5, scalar2=None,
                                op0=_mult)
        ibits = st.tile([P, J], I32, name="ibits")
        nc.vector.tensor_scalar(out=ibits, in0=vpe.bitcast(I32), scalar1=1,
                                scalar2=None,
                                op0=mybir.AluOpType.logical_shift_right)
        seed = st.tile([P, J], I32, name="seed")
        nc.vector.tensor_tensor(out=seed, in0=magic4, in1=ibits, op=_sub)
        y_a = st.tile([P, J], F32, name="y_a")
        y2_a = st.tile([P, J], F32, name="y2_a")
        nc.vector.tensor_tensor(out=y2_a, in0=seed.bitcast(F32),
                                in1=seed.bitcast(F32), op=_mult)
        w_a = st.tile([P, J], F32, name="w_a")
        nc.gpsimd.tensor_tensor(out=w_a, in0=y2_a, in1=nvpe, op=_mult)
        nc.vector.scalar_tensor_tensor(out=y_a, in0=w_a, scalar=1.5,
                                       in1=seed.bitcast(F32), op0=_add,
                                       op1=_mult)
        y2_b = st.tile([P, J], F32, name="y2_b")
        nc.vector.tensor_tensor(out=y2_b, in0=y_a, in1=y_a, op=_mult)
        w_b = st.tile([P, J], F32, name="w_b")
        nc.gpsimd.tensor_tensor(out=w_b, in0=y2_b, in1=nvpe, op=_mult)
        rstd = st.tile([P, J], F32, name="rstd")
        nc.vector.scalar_tensor_tensor(out=rstd, in0=w_b, scalar=1.5,
                                       in1=y_a, op0=_add, op1=_mult)
        nbias = st.tile([P, J], F32, name="nbias")
        nc.vector.scalar_tensor_tensor(out=nbias, in0=mus, scalar=-1.0,
                                       in1=rstd, op0=_mult, op1=_mult)

        # ---------- normalize + silu ----------
        a_sb = wk.tile([P, J, HID], F32R, name="a_sb")
        if not gbe_nz:
            for j in range(J):
                nc.scalar.activation(a_sb[:, j], h_ps[:, j, 0:HID],
                                     mybir.ActivationFunctionType.Silu,
                                     bias=nbias[:, j:j + 1],
                                     scale=rstd[:, j:j + 1])
        else:
            hn = wk.tile([P, J, HID], F32, name="hn")
            for j in range(J):
                nc.scalar.activation(hn[:, j], h_ps[:, j, 0:HID],
                                     mybir.ActivationFunctionType.Identity,
                                     bias=nbias[:, j:j + 1],
                                     scale=rstd[:, j:j + 1])
            hg = wk.tile([P, J, HID], F32, name="hg")
            for j in range(J):
                nc.vector.tensor_tensor(out=hg[:, j], in0=hn[:, j],
                                        in1=grep_sb, op=_mult)
                nc.vector.tensor_tensor(out=hg[:, j], in0=hg[:, j],
                                        in1=berep_sb, op=_add)
            for j in range(J):
                nc.scalar.activation(a_sb[:, j], hg[:, j],
                                     mybir.ActivationFunctionType.Silu)

        # ---------- PE back: transpose a, mm2 (float32r) ----------
        at_ps = ps_at.tile([P, J, HID], F32R, name="at_ps")
        for j in range(J):
            nc.tensor.transpose(at_ps[:, j, :], a_sb[:, j, :], ident_r)
        at_r = wk.tile([P, J, HID], F32R, name="at_r")
        nc.scalar.copy(at_r, at_ps)

        wb = ps_w.tile([P, J, 512], F32, name="wb")
        for j in range(J):
            nc.tensor.matmul(wb[:, j, 0:384], at_r[:, j, :], w2r,
                             start=True, stop=True)

        # ---------- DTP ----------
        out_t = io.tile([P, J, FEA], F32, name="out_t")
        x0 = fea_t[:, :, 0:MUL]                    # [P,J,64]
        x1 = fea_t[:, :, MUL:FEA]                  # [P,J,192]
        vrow = vec_sb[:, m, :].rearrange("p (j f) -> p j f", f=4)   # [P,J,4]

        # t2 = w2' * x0   (PSUM cross-bank read)
        t2 = wk.tile([P, J, MUL], F32, name="t2")
        nc.vector.tensor_tensor(out=t2, in0=wb[:, :, 64:128], in1=x0, op=_mult)

        # E_j = t2 (x) y1 ; G_j = (x1*y0)*w3rep ; B_j = x1*y1
        e_t = wk.tile([P, J, MUL, 3], F32, name="e_t")
        g_t = wk.tile([P, J, MUL * 3], F32, name="g_t")
        b_t = wk.tile([P, J, MUL, 3], F32, name="b_t")
        for j in range(J):
            y1bj = vrow[:, j, 1:4].unsqueeze(1).broadcast_to([P, MUL, 3])
            nc.vector.tensor_tensor(
                out=e_t[:, j],
                in0=t2[:, j, :].unsqueeze(2).broadcast_to([P, MUL, 3]),
                in1=y1bj, op=_mult)
            nc.vector.scalar_tensor_tensor(
                out=g_t[:, j], in0=x1[:, j], scalar=vrow[:, j, 0:1],
                in1=wb[:, j, 128:320], op0=_mult, op1=_mult)
            nc.gpsimd.tensor_tensor(
                out=b_t[:, j],
                in0=x1[:, j].rearrange("p (u d) -> p u d", d=3),
                in1=y1bj, op=_mult)

        # out1 = E + G
        nc.vector.tensor_tensor(
            out=out_t[:, :, MUL:FEA],
            in0=e_t.rearrange("p j u d -> p j (u d)"), in1=g_t, op=_add)

        # D = sum_d B ; m1y = (x0*y0)*w1' ; mD = D*w4' ; out0 = m1y + mD
        d_t = wk.tile([P, J, MUL], F32, name="d_t")
        nc.vector.tensor_reduce(out=d_t, in_=b_t, axis=mybir.AxisListType.X,
                                op=_add)
        p1 = wk.tile([P, J, MUL], F32, name="p1")
        y0b = vrow[:, :, 0:1].broadcast_to([P, J, MUL])
        nc.gpsimd.tensor_tensor(out=p1, in0=x0, in1=y0b, op=_mult)
        m1y = wk.tile([P, J, MUL], F32, name="m1y")
        nc.vector.tensor_tensor(out=m1y, in0=p1, in1=wb[:, :, 0:64], op=_mult)
        md = wk.tile([P, J, MUL], F32, name="md")
        nc.vector.tensor_tensor(out=md, in0=d_t, in1=wb[:, :, 320:384], op=_mult)
        nc.gpsimd.tensor_tensor(out=out_t[:, :, 0:MUL], in0=m1y, in1=md, op=_add)

        # ---------- store ----------
        nc.sync.dma_start(out=out_v[m], in_=out_t)


def _build_program(flags):
    """flags = {key: (b1_nz, gbe_nz)}"""
    import contextlib
    nc = bacc.Bacc("TRN2", target_bir_lowering=False, debug=False)
    with tile.TileContext(nc) as tc:
        with contextlib.ExitStack() as ctx:
            glob = ctx.enter_context(tc.tile_pool(name="glob", bufs=1))
            ident = glob.tile([P, P], F32)
            make_identity(nc, ident)
            ident_r = glob.tile([P, P], F32R)
            nc.scalar.copy(ident_r, ident)
            magic4 = glob.tile([P, J], I32)
            nc.vector.memset(magic4, 0x5F3759DF)
            eps_t = glob.tile([P, 1], F32)
            nc.vector.memset(eps_t, EPS)
            pools = (
                ctx.enter_context(tc.tile_pool(name="io", bufs=3)),
                ctx.enter_context(tc.tile_pool(name="wk", bufs=2)),
                ctx.enter_context(tc.tile_pool(name="st", bufs=2)),
                ctx.enter_context(tc.tile_pool(name="pslt", bufs=1, space="PSUM")),
                ctx.enter_context(tc.tile_pool(name="psh", bufs=1, space="PSUM")),
                ctx.enter_context(tc.tile_pool(name="psat", bufs=1, space="PSUM")),
                ctx.enter_context(tc.tile_pool(name="psw", bufs=1, space="PSUM")),
            )
            for key in ("a", "b"):
                b1_nz, gbe_nz = flags[key]
                _build_key(nc, tc, ctx, key, b1_nz, gbe_nz, ident, ident_r, magic4, pools)
    nc.compile()
    return nc


def kernel(edge_fea_a, edge_vec_a, edge_len_a, W1_a, b1_a, g_a, be_a, W2_a,
           edge_fea_b, edge_vec_b, edge_len_b, W1_b, b1_b, g_b, be_b, W2_b):
    global last_exec_time_ns, last_results
    ins = {
        "a": (edge_fea_a, edge_vec_a, edge_len_a, W1_a, b1_a, g_a, be_a, W2_a),
        "b": (edge_fea_b, edge_vec_b, edge_len_b, W1_b, b1_b, g_b, be_b, W2_b),
    }
    prepped = {}
    flags = {}
    for key, (fea, vec, lng, W1, b1, g, be, W2) in ins.items():
        W1m, W2big, b1_nz = _prep_weights(np.asarray(W1, np.float32),
                                          np.asarray(b1, np.float32),
                                          np.asarray(W2, np.float32))
        gbe_nz = bool(np.any(np.asarray(g) != 1.0) or np.any(np.asarray(be)))
        prepped[key] = (W1m, W2big)
        flags[key] = (b1_nz, gbe_nz)

    ck = tuple(flags[k] for k in ("a", "b"))
    if ck not in _CACHE:
        _CACHE[ck] = _build_program(flags)
    nc = _CACHE[ck]

    in_maps = []
    for c in range(NCORE):
        sl = slice(c * ESH, (c + 1) * ESH)
        m = {}
        for key, (fea, vec, lng, W1, b1, g, be, W2) in ins.items():
            m[f"fea_{key}"] = np.ascontiguousarray(np.asarray(fea, np.float32)[sl])
            m[f"vec_{key}"] = np.ascontiguousarray(np.asarray(vec, np.float32)[sl])
            m[f"len_{key}"] = np.ascontiguousarray(np.asarray(lng, np.float32)[sl])
            m[f"w1m_{key}"] = prepped[key][0]
            m[f"w2b_{key}"] = prepped[key][1]
            if flags[key][1]:
                m[f"g_{key}"] = np.asarray(g, np.float32)
                m[f"be_{key}"] = np.asarray(be, np.float32)
        in_maps.append(m)

    trace = bool(int(os.environ.get("KERNEL_TRACE", "0")))
    res = run_bass_kernel_spmd(nc, in_maps, list(range(NCORE)), trace=trace)
    globals()["last_results"] = res
    last_exec_time_ns = res.exec_time_ns

    out_a = np.concatenate([np.asarray(res.results[c]["out_a"])
                            for c in range(NCORE)], axis=0)
    out_b = np.concatenate([np.asarray(res.results[c]["out_b"])
                            for c in range(NCORE)], axis=0)
    return (out_a, out_b)


# revision 6
# speedup vs baseline: 1.3626x; 1.3626x over previous
"""Trainium2 Bass kernel for DepthwiseTensorProductModuleDict.

Computes, for each key k in {a, b}:
    w = MLP(edge_len_k)           # Linear(64->128) -> LayerNorm -> silu -> Linear(128->256)
    out_k = DTP(edge_fea_k, edge_vec_k, w)   # depthwise uvu tensor product

Sharding: edge dimension split across 8 NeuronCores (pure data parallel),
both dict keys processed by every core on its edge shard. Weights replicated.

Layout: edges packed 4 per partition -> macro tiles of 512 edges
[128 partitions, 4 slots, features]. Per-macro pipeline:
  PE: transpose len -> mm1 (fp32, N=129 with fused mean column) ->
      transpose a -> mm2 (float32r, N=384, host-packed [w1|w2|w3rep|w4])
  ACT: Square+accum (sum h^2), Silu(scale,bias) for layernorm+silu fusion,
       PSUM->SBUF copies (incl. float32r rounding for mm2 operands)
  DVE/GPSIMD: depthwise tensor product elementwise ops
"""
import os
import numpy as np

import concourse.bass as bass
import concourse.tile as tile
from concourse import bacc, mybir
from concourse.bass_utils import run_bass_kernel_spmd
from concourse.masks import make_identity

F32 = mybir.dt.float32
F32R = mybir.dt.float32r
I32 = mybir.dt.int32
P = 128          # partitions
J = 4            # edges per partition
MACRO = P * J    # 512 edges per macro tile
E = 131072       # total edges per key
NCORE = 8
ESH = E // NCORE          # 16384 edges per core per key
NM = ESH // MACRO         # 32 macros per key per core
MUL = 64
FEA = 256
RAD = 64
HID = 128
EPS = 1e-5

_mult = mybir.AluOpType.mult
_add = mybir.AluOpType.add
_sub = mybir.AluOpType.subtract

# cached compiled program (host-side) keyed by (b1_nz, gbe_nz) per key
_CACHE = {}

last_exec_time_ns = None
last_results = None


def _prep_weights(W1, b1, W2):
    """Host-side weight packing.

    Returns W1m [64 or 65, 129], W2big [128, 384], b1_nz flag.
    W1m = [W1; b1?] with extra column = rowwise mean weights (mu fused in mm1).
    W2big columns: [w1*s2 | w2*s2 | w3rep*s2 (each col x3 interleaved) | w4*s2*s3]
    where s2 = 1/sqrt(2), s3 = 1/sqrt(3).
    """
    inv_s2 = np.float32(1.0 / np.sqrt(np.float32(2.0)))
    inv_s3 = np.float32(1.0 / np.sqrt(np.float32(3.0)))
    b1_nz = bool(np.any(b1))
    Wstack = np.vstack([W1, b1[None, :]]) if b1_nz else W1   # [64(65), 128]
    mu_col = Wstack.mean(axis=1, keepdims=True)              # [*, 1]
    pad = np.zeros_like(mu_col)
    W1m = np.ascontiguousarray(Wstack.astype(np.float32))    # [*, 128]
    W1mu = np.hstack([mu_col, pad]).astype(np.float32)       # [*, 2]

    w1 = W2[:, 0:64] * inv_s2
    w2 = W2[:, 64:128] * inv_s2
    w3 = W2[:, 128:192] * inv_s2
    w4 = W2[:, 192:256] * (inv_s2 * inv_s3)
    w3rep = np.repeat(w3, 3, axis=1)                         # [128, 192]
    W2bigA = np.concatenate([w3rep, w4], axis=1).astype(np.float32)   # [128, 256]
    W2bigB = np.concatenate([w1, w2], axis=1).astype(np.float32)      # [128, 128]
    return W1m, W1mu, W2bigA, W2bigB, b1_nz


def _build_key(nc, tc, ctx, key, b1_nz, gbe_nz, ident, ident_r, magic4, pools):
    """Emit instructions for one dict key's full shard (NM macros)."""
    KROWS = 65 if b1_nz else 64

    fea = nc.dram_tensor(f"fea_{key}", [ESH, FEA], F32, kind="ExternalInput").ap()
    vec = nc.dram_tensor(f"vec_{key}", [ESH, 4], F32, kind="ExternalInput").ap()
    lng = nc.dram_tensor(f"len_{key}", [ESH, RAD], F32, kind="ExternalInput").ap()
    w1m_d = nc.dram_tensor(f"w1m_{key}", [KROWS, HID], F32,
                           kind="ExternalInput").ap()
    w1mu_d = nc.dram_tensor(f"w1mu_{key}", [KROWS, 2], F32,
                            kind="ExternalInput").ap()
    w2a_d = nc.dram_tensor(f"w2a_{key}", [HID, 256], F32,
                           kind="ExternalInput").ap()
    w2b_d = nc.dram_tensor(f"w2b_{key}", [HID, HID], F32,
                           kind="ExternalInput").ap()
    out = nc.dram_tensor(f"out_{key}", [ESH, FEA], F32, kind="ExternalOutput").ap()
    g_d = be_d = None
    if gbe_nz:
        g_d = nc.dram_tensor(f"g_{key}", [HID], F32, kind="ExternalInput").ap()
        be_d = nc.dram_tensor(f"be_{key}", [HID], F32, kind="ExternalInput").ap()

    fea_v = fea.rearrange("(m p j) f -> m p j f", p=P, j=J)
    len_v = lng.rearrange("(m p j) f -> m p j f", p=P, j=J)
    out_v = out.rearrange("(m p j) f -> m p j f", p=P, j=J)
    vec_v = vec.rearrange("(m p j) f -> p m (j f)", p=P, j=J)   # [128, NM, 16]

    const = ctx.enter_context(tc.tile_pool(name=f"const_{key}", bufs=1))

    # --- weights ---
    w1m_stage = const.tile([KROWS, HID], F32)
    nc.sync.dma_start(out=w1m_stage, in_=w1m_d)
    w1m_sb = const.tile([KROWS, HID], F32R)
    nc.scalar.copy(w1m_sb, w1m_stage)
    w1mu_stage = const.tile([KROWS, 2], F32)
    nc.sync.dma_start(out=w1mu_stage, in_=w1mu_d)
    w1mu_sb = const.tile([KROWS, 2], F32R)
    nc.scalar.copy(w1mu_sb, w1mu_stage)
    w2a_stage = const.tile([HID, 256], F32)
    nc.sync.dma_start(out=w2a_stage, in_=w2a_d)
    w2ar = const.tile([HID, 256], F32R)
    nc.scalar.copy(w2ar, w2a_stage)
    w2b_stage = const.tile([HID, HID], F32)
    nc.sync.dma_start(out=w2b_stage, in_=w2b_d)
    w2br = const.tile([HID, HID], F32R)
    nc.scalar.copy(w2br, w2b_stage)

    grep_sb = berep_sb = None
    if gbe_nz:
        grep_sb = const.tile([P, HID], F32)
        berep_sb = const.tile([P, HID], F32)
        nc.sync.dma_start(out=grep_sb, in_=g_d.partition_broadcast(P))
        nc.sync.dma_start(out=berep_sb, in_=be_d.partition_broadcast(P))

    # --- whole-shard vec resident in SBUF ---
    vec_sb = const.tile([P, NM, J * 4], F32)
    nc.sync.dma_start(out=vec_sb, in_=vec_v)

    io, wk, st, ps_lt, ps_h, ps_at, ps_wa, ps_wb, ps_mu = pools

    for m in range(NM):
        # ---------- loads ----------
        len_t = io.tile([P, J, RAD], F32, name="len_t")
        nc.sync.dma_start(out=len_t, in_=len_v[m])
        fea_t = io.tile([P, J, FEA], F32, name="fea_t")
        nc.sync.dma_start(out=fea_t, in_=fea_v[m])

        # ---------- PE front: transpose len, mm1 ----------
        lt_ps = ps_lt.tile([RAD, J * P], F32, name="lt_ps")
        for j in range(J):
            nc.tensor.transpose(lt_ps[:, j * P:(j + 1) * P], len_t[:, j, :], ident)
        lt_sb = wk.tile([KROWS, J * P], F32R, name="lt_sb")
        nc.scalar.copy(lt_sb[0:RAD, :], lt_ps)
        if b1_nz:
            nc.gpsimd.memset(lt_sb[RAD:KROWS, :], 1.0)

        h_ps = ps_h.tile([P, J, HID], F32, name="h_ps")
        mu_ps = ps_mu.tile([P, J, 2], F32, name="mu_ps")
        for j in range(J):
            nc.tensor.matmul(h_ps[:, j, :],
                             lt_sb[:, j * P:(j + 1) * P], w1m_sb,
                             start=True, stop=True)
            nc.tensor.matmul(mu_ps[:, j, :],
                             lt_sb[:, j * P:(j + 1) * P], w1mu_sb,
                             start=True, stop=True)

        # ---------- layernorm stats ----------
        sq_d = wk.tile([P, J, HID], F32, name="sq_d")
        ssq = st.tile([P, J], F32, name="ssq")
        for j in range(J):
            nc.scalar.activation(sq_d[:, j], h_ps[:, j, :],
                                 mybir.ActivationFunctionType.Square,
                                 accum_out=ssq[:, j:j + 1])
        mus = st.tile([P, J], F32, name="mus")
        nc.vector.tensor_copy(mus, mu_ps[:, :, 0:1].squeeze(2))

        # var = ssq/128 - mus^2 ; rstd = 1/sqrt(var+eps); nbias = -mus*rstd
        musq = st.tile([P, J], F32, name="musq")
        nc.gpsimd.tensor_tensor(out=musq, in0=mus, in1=mus, op=_mult)
        var = st.tile([P, J], F32, name="var")
        nc.vector.scalar_tensor_tensor(out=var, in0=ssq, scalar=1.0 / HID,
                                       in1=musq, op0=_mult, op1=_sub)
        vpe = st.tile([P, J], F32, name="vpe")
        nc.vector.tensor_scalar(out=vpe, in0=var, scalar1=EPS, scalar2=None,
                                op0=_add)
        nvpe = st.tile([P, J], F32, name="nvpe")
        nc.vector.tensor_scalar(out=nvpe, in0=vpe, scalar1=-0.5, scalar2=None,
                                op0=_mult)
        ibits = st.tile([P, J], I32, name="ibits")
        nc.vector.tensor_scalar(out=ibits, in0=vpe.bitcast(I32), scalar1=1,
                                scalar2=None,
                                op0=mybir.AluOpType.logical_shift_right)
        seed = st.tile([P, J], I32, name="seed")
        nc.vector.tensor_tensor(out=seed, in0=magic4, in1=ibits, op=_sub)
        y_a = st.tile([P, J], F32, name="y_a")
        y2_a = st.tile([P, J], F32, name="y2_a")
        nc.vector.tensor_tensor(out=y2_a, in0=seed.bitcast(F32),
                                in1=seed.bitcast(F32), op=_mult)
        w_a = st.tile([P, J], F32, name="w_a")
        nc.gpsimd.tensor_tensor(out=w_a, in0=y2_a, in1=nvpe, op=_mult)
        nc.vector.scalar_tensor_tensor(out=y_a, in0=w_a, scalar=1.5,
                                       in1=seed.bitcast(F32), op0=_add,
                                       op1=_mult)
        y2_b = st.tile([P, J], F32, name="y2_b")
        nc.vector.tensor_tensor(out=y2_b, in0=y_a, in1=y_a, op=_mult)
        w_b = st.tile([P, J], F32, name="w_b")
        nc.gpsimd.tensor_tensor(out=w_b, in0=y2_b, in1=nvpe, op=_mult)
        rstd = st.tile([P, J], F32, name="rstd")
        nc.vector.scalar_tensor_tensor(out=rstd, in0=w_b, scalar=1.5,
                                       in1=y_a, op0=_add, op1=_mult)
        nbias = st.tile([P, J], F32, name="nbias")
        nc.vector.scalar_tensor_tensor(out=nbias, in0=mus, scalar=-1.0,
                                       in1=rstd, op0=_mult, op1=_mult)

        # ---------- normalize + silu ----------
        a_sb = wk.tile([P, J, HID], F32R, name="a_sb")
        if not gbe_nz:
            for j in range(J):
                nc.scalar.activation(a_sb[:, j], h_ps[:, j, :],
                                     mybir.ActivationFunctionType.Silu,
                                     bias=nbias[:, j:j + 1],
                                     scale=rstd[:, j:j + 1])
        else:
            hn = wk.tile([P, J, HID], F32, name="hn")
            for j in range(J):
                nc.scalar.activation(hn[:, j], h_ps[:, j, :],
                                     mybir.ActivationFunctionType.Identity,
                                     bias=nbias[:, j:j + 1],
                                     scale=rstd[:, j:j + 1])
            hg = wk.tile([P, J, HID], F32, name="hg")
            for j in range(J):
                nc.vector.tensor_tensor(out=hg[:, j], in0=hn[:, j],
                                        in1=grep_sb, op=_mult)
                nc.vector.tensor_tensor(out=hg[:, j], in0=hg[:, j],
                                        in1=berep_sb, op=_add)
            for j in range(J):
                nc.scalar.activation(a_sb[:, j], hg[:, j],
                                     mybir.ActivationFunctionType.Silu)

        # ---------- PE back: transpose a, mm2 (float32r) ----------
        at_ps = ps_at.tile([P, J, HID], F32R, name="at_ps")
        for j in range(J):
            nc.tensor.transpose(at_ps[:, j, :], a_sb[:, j, :], ident_r)
        at_r = wk.tile([P, J, HID], F32R, name="at_r")
        nc.scalar.copy(at_r, at_ps)

        wba = ps_wa.tile([P, J, 256], F32, name="wba")   # [w3rep|w4]
        wbb = ps_wb.tile([P, J, HID], F32, name="wbb")   # [w1|w2]
        for j in range(J):
            nc.tensor.matmul(wba[:, j, :], at_r[:, j, :], w2ar,
                             start=True, stop=True)
            nc.tensor.matmul(wbb[:, j, :], at_r[:, j, :], w2br,
                             start=True, stop=True)

        # ---------- DTP ----------
        out_t = io.tile([P, J, FEA], F32, name="out_t")
        x0 = fea_t[:, :, 0:MUL]                    # [P,J,64]
        x1 = fea_t[:, :, MUL:FEA]                  # [P,J,192]
        vrow = vec_sb[:, m, :].rearrange("p (j f) -> p j f", f=4)   # [P,J,4]

        # t2 = w2' * x0   (PSUM cross-bank read)
        t2 = wk.tile([P, J, MUL], F32, name="t2")
        nc.vector.tensor_tensor(out=t2, in0=wbb[:, :, 64:128], in1=x0, op=_mult)

        # E_j = t2 (x) y1 ; G_j = (x1*y0)*w3rep ; B_j = x1*y1
        e_t = wk.tile([P, J, MUL, 3], F32, name="e_t")
        g_t = wk.tile([P, J, MUL * 3], F32, name="g_t")
        b_t = wk.tile([P, J, MUL, 3], F32, name="b_t")
        for j in range(J):
            y1bj = vrow[:, j, 1:4].unsqueeze(1).broadcast_to([P, MUL, 3])
            eng = nc.gpsimd if j < 2 else nc.vector
            eng.tensor_tensor(
                out=e_t[:, j],
                in0=t2[:, j, :].unsqueeze(2).broadcast_to([P, MUL, 3]),
                in1=y1bj, op=_mult)
            nc.vector.scalar_tensor_tensor(
                out=g_t[:, j], in0=x1[:, j], scalar=vrow[:, j, 0:1],
                in1=wba[:, j, 0:192], op0=_mult, op1=_mult)
            nc.gpsimd.tensor_tensor(
                out=b_t[:, j],
                in0=x1[:, j].rearrange("p (u d) -> p u d", d=3),
                in1=y1bj, op=_mult)

        # out1 = E + G
        nc.vector.tensor_tensor(
            out=out_t[:, :, MUL:FEA],
            in0=e_t.rearrange("p j u d -> p j (u d)"), in1=g_t, op=_add)

        # D = sum_d B ; m1y = (x0*y0)*w1' ; mD = D*w4' ; out0 = m1y + mD
        d_t = wk.tile([P, J, MUL], F32, name="d_t")
        nc.vector.tensor_reduce(out=d_t, in_=b_t, axis=mybir.AxisListType.X,
                                op=_add)
        p1 = wk.tile([P, J, MUL], F32, name="p1")
        y0b = vrow[:, :, 0:1].broadcast_to([P, J, MUL])
        nc.gpsimd.tensor_tensor(out=p1, in0=x0, in1=y0b, op=_mult)
        m1y = wk.tile([P, J, MUL], F32, name="m1y")
        nc.vector.tensor_tensor(out=m1y, in0=p1, in1=wbb[:, :, 0:64], op=_mult)
        md = wk.tile([P, J, MUL], F32, name="md")
        nc.vector.tensor_tensor(out=md, in0=d_t, in1=wba[:, :, 192:256], op=_mult)
        nc.gpsimd.tensor_tensor(out=out_t[:, :, 0:MUL], in0=m1y, in1=md, op=_add)

        # ---------- store ----------
        nc.sync.dma_start(out=out_v[m], in_=out_t)


def _build_program(flags):
    """flags = {key: (b1_nz, gbe_nz)}"""
    import contextlib
    nc = bacc.Bacc("TRN2", target_bir_lowering=False, debug=False)
    with tile.TileContext(nc) as tc:
        with contextlib.ExitStack() as ctx:
            glob = ctx.enter_context(tc.tile_pool(name="glob", bufs=1))
            ident = glob.tile([P, P], F32)
            make_identity(nc, ident)
            ident_r = glob.tile([P, P], F32R)
            nc.scalar.copy(ident_r, ident)
            magic4 = glob.tile([P, J], I32)
            nc.vector.memset(magic4, 0x5F3759DF)
            eps_t = glob.tile([P, 1], F32)
            nc.vector.memset(eps_t, EPS)
            pools = (
                ctx.enter_context(tc.tile_pool(name="io", bufs=3)),
                ctx.enter_context(tc.tile_pool(name="wk", bufs=2)),
                ctx.enter_context(tc.tile_pool(name="st", bufs=2)),
                ctx.enter_context(tc.tile_pool(name="pslt", bufs=1, space="PSUM")),
                ctx.enter_context(tc.tile_pool(name="psh", bufs=2, space="PSUM")),
                ctx.enter_context(tc.tile_pool(name="psat", bufs=1, space="PSUM")),
                ctx.enter_context(tc.tile_pool(name="pswa", bufs=1, space="PSUM")),
                ctx.enter_context(tc.tile_pool(name="pswb", bufs=1, space="PSUM")),
                ctx.enter_context(tc.tile_pool(name="psmu", bufs=1, space="PSUM")),
            )
            for key in ("a", "b"):
                b1_nz, gbe_nz = flags[key]
                _build_key(nc, tc, ctx, key, b1_nz, gbe_nz, ident, ident_r, magic4, pools)
    nc.compile()
    return nc


def kernel(edge_fea_a, edge_vec_a, edge_len_a, W1_a, b1_a, g_a, be_a, W2_a,
           edge_fea_b, edge_vec_b, edge_len_b, W1_b, b1_b, g_b, be_b, W2_b):
    global last_exec_time_ns, last_results
    ins = {
        "a": (edge_fea_a, edge_vec_a, edge_len_a, W1_a, b1_a, g_a, be_a, W2_a),
        "b": (edge_fea_b, edge_vec_b, edge_len_b, W1_b, b1_b, g_b, be_b, W2_b),
    }
    prepped = {}
    flags = {}
    for key, (fea, vec, lng, W1, b1, g, be, W2) in ins.items():
        W1m, W1mu, W2bigA, W2bigB, b1_nz = _prep_weights(
            np.asarray(W1, np.float32), np.asarray(b1, np.float32),
            np.asarray(W2, np.float32))
        gbe_nz = bool(np.any(np.asarray(g) != 1.0) or np.any(np.asarray(be)))
        prepped[key] = (W1m, W1mu, W2bigA, W2bigB)
        flags[key] = (b1_nz, gbe_nz)

    ck = tuple(flags[k] for k in ("a", "b"))
    if ck not in _CACHE:
        _CACHE[ck] = _build_program(flags)
    nc = _CACHE[ck]

    in_maps = []
    for c in range(NCORE):
        sl = slice(c * ESH, (c + 1) * ESH)
        m = {}
        for key, (fea, vec, lng, W1, b1, g, be, W2) in ins.items():
            m[f"fea_{key}"] = np.ascontiguousarray(np.asarray(fea, np.float32)[sl])
            m[f"vec_{key}"] = np.ascontiguousarray(np.asarray(vec, np.float32)[sl])
            m[f"len_{key}"] = np.ascontiguousarray(np.asarray(lng, np.float32)[sl])
            m[f"w1m_{key}"] = prepped[key][0]
            m[f"w1mu_{key}"] = prepped[key][1]
            m[f"w2a_{key}"] = prepped[key][2]
            m[f"w2b_{key}"] = prepped[key][3]
            if flags[key][1]:
                m[f"g_{key}"] = np.asarray(g, np.float32)
                m[f"be_{key}"] = np.asarray(be, np.float32)
        in_maps.append(m)

    trace = bool(int(os.environ.get("KERNEL_TRACE", "0")))
    res = run_bass_kernel_spmd(nc, in_maps, list(range(NCORE)), trace=trace)
    globals()["last_results"] = res
    last_exec_time_ns = res.exec_time_ns

    out_a = np.concatenate([np.asarray(res.results[c]["out_a"])
                            for c in range(NCORE)], axis=0)
    out_b = np.concatenate([np.asarray(res.results[c]["out_b"])
                            for c in range(NCORE)], axis=0)
    return (out_a, out_b)


# revision 7
# speedup vs baseline: 1.3790x; 1.0120x over previous
"""Trainium2 Bass kernel for DepthwiseTensorProductModuleDict.

Computes, for each key k in {a, b}:
    w = MLP(edge_len_k)           # Linear(64->128) -> LayerNorm -> silu -> Linear(128->256)
    out_k = DTP(edge_fea_k, edge_vec_k, w)   # depthwise uvu tensor product

Sharding: edge dimension split across 8 NeuronCores (pure data parallel),
both dict keys processed by every core on its edge shard. Weights replicated.

Layout: edges packed 4 per partition -> macro tiles of 512 edges
[128 partitions, 4 slots, features]. Per-macro pipeline:
  PE: transpose len -> mm1 (fp32, N=129 with fused mean column) ->
      transpose a -> mm2 (float32r, N=384, host-packed [w1|w2|w3rep|w4])
  ACT: Square+accum (sum h^2), Silu(scale,bias) for layernorm+silu fusion,
       PSUM->SBUF copies (incl. float32r rounding for mm2 operands)
  DVE/GPSIMD: depthwise tensor product elementwise ops
"""
import os
import numpy as np

import concourse.bass as bass
import concourse.tile as tile
from concourse import bacc, mybir
from concourse.bass_utils import run_bass_kernel_spmd
from concourse.masks import make_identity

F32 = mybir.dt.float32
F32R = mybir.dt.float32r
I32 = mybir.dt.int32
P = 128          # partitions
J = 4            # edges per partition
MACRO = P * J    # 512 edges per macro tile
E = 131072       # total edges per key
NCORE = 8
ESH = E // NCORE          # 16384 edges per core per key
NM = ESH // MACRO         # 32 macros per key per core
MUL = 64
FEA = 256
RAD = 64
HID = 128
EPS = 1e-5

_mult = mybir.AluOpType.mult
_add = mybir.AluOpType.add
_sub = mybir.AluOpType.subtract

# cached compiled program (host-side) keyed by (b1_nz, gbe_nz) per key
_CACHE = {}

last_exec_time_ns = None
last_results = None


def _prep_weights(W1, b1, W2):
    """Host-side weight packing.

    Returns W1m [64 or 65, 129], W2big [128, 384], b1_nz flag.
    W1m = [W1; b1?] with extra column = rowwise mean weights (mu fused in mm1).
    W2big columns: [w1*s2 | w2*s2 | w3rep*s2 (each col x3 interleaved) | w4*s2*s3]
    where s2 = 1/sqrt(2), s3 = 1/sqrt(3).
    """
    inv_s2 = np.float32(1.0 / np.sqrt(np.float32(2.0)))
    inv_s3 = np.float32(1.0 / np.sqrt(np.float32(3.0)))
    b1_nz = bool(np.any(b1))
    Wstack = np.vstack([W1, b1[None, :]]) if b1_nz else W1   # [64(65), 128]
    mu_col = Wstack.mean(axis=1, keepdims=True)              # [*, 1]
    pad = np.zeros_like(mu_col)
    W1m = np.ascontiguousarray(Wstack.astype(np.float32))    # [*, 128]
    W1mu = np.hstack([mu_col, pad]).astype(np.float32)       # [*, 2]

    w1 = W2[:, 0:64] * inv_s2
    w2 = W2[:, 64:128] * inv_s2
    w3 = W2[:, 128:192] * inv_s2
    w4 = W2[:, 192:256] * (inv_s2 * inv_s3)
    w3rep = np.repeat(w3, 3, axis=1)                         # [128, 192]
    W2bigA = np.concatenate([w3rep, w4], axis=1).astype(np.float32)   # [128, 256]
    W2bigB = np.concatenate([w1, w2], axis=1).astype(np.float32)      # [128, 128]
    return W1m, W1mu, W2bigA, W2bigB, b1_nz


def _build_key(nc, tc, ctx, key, b1_nz, gbe_nz, ident, ident_r, magic4, pools):
    """Emit instructions for one dict key's full shard (NM macros)."""
    KROWS = 65 if b1_nz else 64

    fea = nc.dram_tensor(f"fea_{key}", [ESH, FEA], F32, kind="ExternalInput").ap()
    vec = nc.dram_tensor(f"vec_{key}", [ESH, 4], F32, kind="ExternalInput").ap()
    lng = nc.dram_tensor(f"len_{key}", [ESH, RAD], F32, kind="ExternalInput").ap()
    w1m_d = nc.dram_tensor(f"w1m_{key}", [KROWS, HID], F32,
                           kind="ExternalInput").ap()
    w1mu_d = nc.dram_tensor(f"w1mu_{key}", [KROWS, 2], F32,
                            kind="ExternalInput").ap()
    w2a_d = nc.dram_tensor(f"w2a_{key}", [HID, 256], F32,
                           kind="ExternalInput").ap()
    w2b_d = nc.dram_tensor(f"w2b_{key}", [HID, HID], F32,
                           kind="ExternalInput").ap()
    out = nc.dram_tensor(f"out_{key}", [ESH, FEA], F32, kind="ExternalOutput").ap()
    g_d = be_d = None
    if gbe_nz:
        g_d = nc.dram_tensor(f"g_{key}", [HID], F32, kind="ExternalInput").ap()
        be_d = nc.dram_tensor(f"be_{key}", [HID], F32, kind="ExternalInput").ap()

    fea_v = fea.rearrange("(m p j) f -> m p j f", p=P, j=J)
    len_v = lng.rearrange("(m p j) f -> m p j f", p=P, j=J)
    out_v = out.rearrange("(m p j) f -> m p j f", p=P, j=J)
    vec_v = vec.rearrange("(m p j) f -> p m (j f)", p=P, j=J)   # [128, NM, 16]

    const = ctx.enter_context(tc.tile_pool(name=f"const_{key}", bufs=1))

    # --- weights ---
    w1m_stage = const.tile([KROWS, HID], F32)
    nc.sync.dma_start(out=w1m_stage, in_=w1m_d)
    w1m_sb = const.tile([KROWS, HID], F32R)
    nc.scalar.copy(w1m_sb, w1m_stage)
    w1mu_stage = const.tile([KROWS, 2], F32)
    nc.sync.dma_start(out=w1mu_stage, in_=w1mu_d)
    w1mu_sb = const.tile([KROWS, 2], F32R)
    nc.scalar.copy(w1mu_sb, w1mu_stage)
    w2a_stage = const.tile([HID, 256], F32)
    nc.sync.dma_start(out=w2a_stage, in_=w2a_d)
    w2ar = const.tile([HID, 256], F32R)
    nc.scalar.copy(w2ar, w2a_stage)
    w2b_stage = const.tile([HID, HID], F32)
    nc.sync.dma_start(out=w2b_stage, in_=w2b_d)
    w2br = const.tile([HID, HID], F32R)
    nc.scalar.copy(w2br, w2b_stage)

    grep_sb = berep_sb = None
    if gbe_nz:
        grep_sb = const.tile([P, HID], F32)
        berep_sb = const.tile([P, HID], F32)
        nc.sync.dma_start(out=grep_sb, in_=g_d.partition_broadcast(P))
        nc.sync.dma_start(out=berep_sb, in_=be_d.partition_broadcast(P))

    # --- whole-shard vec resident in SBUF ---
    vec_sb = const.tile([P, NM, J * 4], F32)
    nc.sync.dma_start(out=vec_sb, in_=vec_v)

    io, wk, st, ps_lt, ps_h, ps_at, ps_wa, ps_wb, ps_mu = pools

    for m in range(NM):
        # ---------- loads ----------
        len_t = io.tile([P, J, RAD], F32, name="len_t")
        nc.sync.dma_start(out=len_t, in_=len_v[m])
        fea_t = io.tile([P, J, FEA], F32, name="fea_t")
        nc.sync.dma_start(out=fea_t, in_=fea_v[m])

        # ---------- PE front: transpose len, mm1 ----------
        lt_ps = ps_lt.tile([RAD, J * P], F32, name="lt_ps")
        for j in range(J):
            nc.tensor.transpose(lt_ps[:, j * P:(j + 1) * P], len_t[:, j, :], ident)
        lt_sb = wk.tile([KROWS, J * P], F32R, name="lt_sb")
        nc.scalar.copy(lt_sb[0:RAD, :], lt_ps)
        if b1_nz:
            nc.gpsimd.memset(lt_sb[RAD:KROWS, :], 1.0)

        h_ps = ps_h.tile([P, J, HID], F32, name="h_ps")
        mu_ps = ps_mu.tile([P, J, 2], F32, name="mu_ps")
        for j in range(J):
            nc.tensor.matmul(h_ps[:, j, :],
                             lt_sb[:, j * P:(j + 1) * P], w1m_sb,
                             start=True, stop=True)
            nc.tensor.matmul(mu_ps[:, j, :],
                             lt_sb[:, j * P:(j + 1) * P], w1mu_sb,
                             start=True, stop=True)

        # ---------- layernorm stats ----------
        sq_d = wk.tile([P, J, HID], F32, name="sq_d")
        ssq = st.tile([P, J], F32, name="ssq")
        for j in range(J):
            nc.scalar.activation(sq_d[:, j], h_ps[:, j, :],
                                 mybir.ActivationFunctionType.Square,
                                 accum_out=ssq[:, j:j + 1])
        mus = st.tile([P, J], F32, name="mus")
        nc.vector.tensor_copy(mus, mu_ps[:, :, 0:1].squeeze(2))

        # var = ssq/128 - mus^2 ; rstd = 1/sqrt(var+eps); nbias = -mus*rstd
        musq = st.tile([P, J], F32, name="musq")
        nc.gpsimd.tensor_tensor(out=musq, in0=mus, in1=mus, op=_mult)
        var = st.tile([P, J], F32, name="var")
        nc.vector.scalar_tensor_tensor(out=var, in0=ssq, scalar=1.0 / HID,
                                       in1=musq, op0=_mult, op1=_sub)
        vpe = st.tile([P, J], F32, name="vpe")
        nc.vector.tensor_scalar(out=vpe, in0=var, scalar1=EPS, scalar2=None,
                                op0=_add)
        nvpe = st.tile([P, J], F32, name="nvpe")
        nc.vector.tensor_scalar(out=nvpe, in0=var, scalar1=-0.5, scalar2=-EPS / 2,
                                op0=_mult, op1=_add)
        ibits = st.tile([P, J], I32, name="ibits")
        nc.vector.tensor_scalar(out=ibits, in0=vpe.bitcast(I32), scalar1=1,
                                scalar2=None,
                                op0=mybir.AluOpType.logical_shift_right)
        seed = st.tile([P, J], I32, name="seed")
        nc.vector.tensor_tensor(out=seed, in0=magic4, in1=ibits, op=_sub)
        y_a = st.tile([P, J], F32, name="y_a")
        y2_a = st.tile([P, J], F32, name="y2_a")
        nc.gpsimd.tensor_tensor(out=y2_a, in0=seed.bitcast(F32),
                                in1=seed.bitcast(F32), op=_mult)
        w_a = st.tile([P, J], F32, name="w_a")
        nc.gpsimd.tensor_tensor(out=w_a, in0=y2_a, in1=nvpe, op=_mult)
        nc.vector.scalar_tensor_tensor(out=y_a, in0=w_a, scalar=1.5,
                                       in1=seed.bitcast(F32), op0=_add,
                                       op1=_mult)
        y2_b = st.tile([P, J], F32, name="y2_b")
        nc.gpsimd.tensor_tensor(out=y2_b, in0=y_a, in1=y_a, op=_mult)
        w_b = st.tile([P, J], F32, name="w_b")
        nc.gpsimd.tensor_tensor(out=w_b, in0=y2_b, in1=nvpe, op=_mult)
        rstd = st.tile([P, J], F32, name="rstd")
        nc.vector.scalar_tensor_tensor(out=rstd, in0=w_b, scalar=1.5,
                                       in1=y_a, op0=_add, op1=_mult)
        nbias = st.tile([P, J], F32, name="nbias")
        nc.vector.scalar_tensor_tensor(out=nbias, in0=mus, scalar=-1.0,
                                       in1=rstd, op0=_mult, op1=_mult)

        # ---------- normalize + silu ----------
        a_sb = wk.tile([P, J, HID], F32R, name="a_sb")
        if not gbe_nz:
            for j in range(J):
                nc.scalar.activation(a_sb[:, j], h_ps[:, j, :],
                                     mybir.ActivationFunctionType.Silu,
                                     bias=nbias[:, j:j + 1],
                                     scale=rstd[:, j:j + 1])
        else:
            hn = wk.tile([P, J, HID], F32, name="hn")
            for j in range(J):
                nc.scalar.activation(hn[:, j], h_ps[:, j, :],
                                     mybir.ActivationFunctionType.Identity,
                                     bias=nbias[:, j:j + 1],
                                     scale=rstd[:, j:j + 1])
            hg = wk.tile([P, J, HID], F32, name="hg")
            for j in range(J):
                nc.vector.tensor_tensor(out=hg[:, j], in0=hn[:, j],
                                        in1=grep_sb, op=_mult)
                nc.vector.tensor_tensor(out=hg[:, j], in0=hg[:, j],
                                        in1=berep_sb, op=_add)
            for j in range(J):
                nc.scalar.activation(a_sb[:, j], hg[:, j],
                                     mybir.ActivationFunctionType.Silu)

        # ---------- PE back: transpose a, mm2 (float32r) ----------
        at_ps = ps_at.tile([P, J, HID], F32R, name="at_ps")
        for j in range(J):
            nc.tensor.transpose(at_ps[:, j, :], a_sb[:, j, :], ident_r)
        at_r = wk.tile([P, J, HID], F32R, name="at_r")
        nc.scalar.copy(at_r, at_ps)

        wba = ps_wa.tile([P, J, 256], F32, name="wba")   # [w3rep|w4]
        wbb = ps_wb.tile([P, J, HID], F32, name="wbb")   # [w1|w2]
        for j in range(J):
            nc.tensor.matmul(wba[:, j, :], at_r[:, j, :], w2ar,
                             start=True, stop=True)
            nc.tensor.matmul(wbb[:, j, :], at_r[:, j, :], w2br,
                             start=True, stop=True)

        # ---------- DTP ----------
        out_t = io.tile([P, J, FEA], F32, name="out_t")
        x0 = fea_t[:, :, 0:MUL]                    # [P,J,64]
        x1 = fea_t[:, :, MUL:FEA]                  # [P,J,192]
        vrow = vec_sb[:, m, :].rearrange("p (j f) -> p j f", f=4)   # [P,J,4]

        # t2 = w2' * x0   (PSUM cross-bank read)
        t2 = wk.tile([P, J, MUL], F32, name="t2")
        nc.vector.tensor_tensor(out=t2, in0=wbb[:, :, 64:128], in1=x0, op=_mult)

        # E_j = t2 (x) y1 ; G_j = (x1*y0)*w3rep ; B_j = x1*y1
        e_t = wk.tile([P, J, MUL, 3], F32, name="e_t")
        g_t = wk.tile([P, J, MUL * 3], F32, name="g_t")
        b_t = wk.tile([P, J, MUL, 3], F32, name="b_t")
        for j in range(J):
            y1bj = vrow[:, j, 1:4].unsqueeze(1).broadcast_to([P, MUL, 3])
            eng = nc.gpsimd if j < 2 else nc.vector
            eng.tensor_tensor(
                out=e_t[:, j],
                in0=t2[:, j, :].unsqueeze(2).broadcast_to([P, MUL, 3]),
                in1=y1bj, op=_mult)
            nc.vector.scalar_tensor_tensor(
                out=g_t[:, j], in0=x1[:, j], scalar=vrow[:, j, 0:1],
                in1=wba[:, j, 0:192], op0=_mult, op1=_mult)
            nc.gpsimd.tensor_tensor(
                out=b_t[:, j],
                in0=x1[:, j].rearrange("p (u d) -> p u d", d=3),
                in1=y1bj, op=_mult)

        # out1 = E + G
        nc.vector.tensor_tensor(
            out=out_t[:, :, MUL:FEA],
            in0=e_t.rearrange("p j u d -> p j (u d)"), in1=g_t, op=_add)

        # D = sum_d B ; m1y = (x0*y0)*w1' ; mD = D*w4' ; out0 = m1y + mD
        d_t = wk.tile([P, J, MUL], F32, name="d_t")
        nc.gpsimd.tensor_tensor(out=d_t, in0=b_t[:, :, :, 0],
                                in1=b_t[:, :, :, 1], op=_add)
        nc.gpsimd.tensor_tensor(out=d_t, in0=d_t,
                                in1=b_t[:, :, :, 2], op=_add)
        p1 = wk.tile([P, J, MUL], F32, name="p1")
        y0b = vrow[:, :, 0:1].broadcast_to([P, J, MUL])
        nc.gpsimd.tensor_tensor(out=p1, in0=x0, in1=y0b, op=_mult)
        m1y = wk.tile([P, J, MUL], F32, name="m1y")
        nc.vector.tensor_tensor(out=m1y, in0=p1, in1=wbb[:, :, 0:64], op=_mult)
        md = wk.tile([P, J, MUL], F32, name="md")
        nc.vector.tensor_tensor(out=md, in0=d_t, in1=wba[:, :, 192:256], op=_mult)
        nc.gpsimd.tensor_tensor(out=out_t[:, :, 0:MUL], in0=m1y, in1=md, op=_add)

        # ---------- store ----------
        nc.sync.dma_start(out=out_v[m], in_=out_t)


def _build_program(flags):
    """flags = {key: (b1_nz, gbe_nz)}"""
    import contextlib
    nc = bacc.Bacc("TRN2", target_bir_lowering=False, debug=False)
    with tile.TileContext(nc) as tc:
        with contextlib.ExitStack() as ctx:
            glob = ctx.enter_context(tc.tile_pool(name="glob", bufs=1))
            ident = glob.tile([P, P], F32)
            make_identity(nc, ident)
            ident_r = glob.tile([P, P], F32R)
            nc.scalar.copy(ident_r, ident)
            magic4 = glob.tile([P, J], I32)
            nc.vector.memset(magic4, 0x5F3759DF)
            eps_t = glob.tile([P, 1], F32)
            nc.vector.memset(eps_t, EPS)
            pools = (
                ctx.enter_context(tc.tile_pool(name="io", bufs=3)),
                ctx.enter_context(tc.tile_pool(name="wk", bufs=2)),
                ctx.enter_context(tc.tile_pool(name="st", bufs=2)),
                ctx.enter_context(tc.tile_pool(name="pslt", bufs=1, space="PSUM")),
                ctx.enter_context(tc.tile_pool(name="psh", bufs=2, space="PSUM")),
                ctx.enter_context(tc.tile_pool(name="psat", bufs=1, space="PSUM")),
                ctx.enter_context(tc.tile_pool(name="pswa", bufs=1, space="PSUM")),
                ctx.enter_context(tc.tile_pool(name="pswb", bufs=1, space="PSUM")),
                ctx.enter_context(tc.tile_pool(name="psmu", bufs=1, space="PSUM")),
            )
            for key in ("a", "b"):
                b1_nz, gbe_nz = flags[key]
                _build_key(nc, tc, ctx, key, b1_nz, gbe_nz, ident, ident_r, magic4, pools)
    nc.compile()
    return nc


def kernel(edge_fea_a, edge_vec_a, edge_len_a, W1_a, b1_a, g_a, be_a, W2_a,
           edge_fea_b, edge_vec_b, edge_len_b, W1_b, b1_b, g_b, be_b, W2_b):
    global last_exec_time_ns, last_results
    ins = {
        "a": (edge_fea_a, edge_vec_a, edge_len_a, W1_a, b1_a, g_a, be_a, W2_a),
        "b": (edge_fea_b, edge_vec_b, edge_len_b, W1_b, b1_b, g_b, be_b, W2_b),
    }
    prepped = {}
    flags = {}
    for key, (fea, vec, lng, W1, b1, g, be, W2) in ins.items():
        W1m, W1mu, W2bigA, W2bigB, b1_nz = _prep_weights(
            np.asarray(W1, np.float32), np.asarray(b1, np.float32),
            np.asarray(W2, np.float32))
        gbe_nz = bool(np.any(np.asarray(g) != 1.0) or np.any(np.asarray(be)))
        prepped[key] = (W1m, W1mu, W2bigA, W2bigB)
        flags[key] = (b1_nz, gbe_nz)

    ck = tuple(flags[k] for k in ("a", "b"))
    if ck not in _CACHE:
        _CACHE[ck] = _build_program(flags)
    nc = _CACHE[ck]

    in_maps = []
    for c in range(NCORE):
        sl = slice(c * ESH, (c + 1) * ESH)
        m = {}
        for key, (fea, vec, lng, W1, b1, g, be, W2) in ins.items():
            m[f"fea_{key}"] = np.ascontiguousarray(np.asarray(fea, np.float32)[sl])
            m[f"vec_{key}"] = np.ascontiguousarray(np.asarray(vec, np.float32)[sl])
            m[f"len_{key}"] = np.ascontiguousarray(np.asarray(lng, np.float32)[sl])
            m[f"w1m_{key}"] = prepped[key][0]
            m[f"w1mu_{key}"] = prepped[key][1]
            m[f"w2a_{key}"] = prepped[key][2]
            m[f"w2b_{key}"] = prepped[key][3]
            if flags[key][1]:
                m[f"g_{key}"] = np.asarray(g, np.float32)
                m[f"be_{key}"] = np.asarray(be, np.float32)
        in_maps.append(m)

    trace = bool(int(os.environ.get("KERNEL_TRACE", "0")))
    res = run_bass_kernel_spmd(nc, in_maps, list(range(NCORE)), trace=trace)
    globals()["last_results"] = res
    last_exec_time_ns = res.exec_time_ns

    out_a = np.concatenate([np.asarray(res.results[c]["out_a"])
                            for c in range(NCORE)], axis=0)
    out_b = np.concatenate([np.asarray(res.results[c]["out_b"])
                            for c in range(NCORE)], axis=0)
    return (out_a, out_b)
